# revision 60
# baseline (speedup 1.0000x reference)
"""ConvGRU Trainium2 kernel (8 NeuronCores, data-parallel over batch x H-half).

Layout (per core shard: batch b, H rows [hh*28, hh*28+28), N = 28*56 = 1568 pixels):
  Channels (C=192) live on SBUF partitions, pixels on the free dim.
  K-stacking for the 1x1 convs (contraction over 2C=384 channels) uses three
  full 128-row SBUF tiles per gate input:
    zr-gates rhs:  H0 = h[0:128] | M1 = [x[128:192]; h[128:192]] | X0 = x[0:128]
    c-gate  rhs:  RH0 = rh[0:128] | M2 = [x[128:192]; rh[128:192]] | X0
  Fused z/r matmul output (o = [z;r], 384) uses M-tiles:
    m0 = z[0:128], m1 = r[0:128], m2 = [z[128:192]; r[128:192]]
  so r[128:192] lands on PSUM/SBUF partitions 64..127, aligned with
  h[128:192] in M1[64:128] for the r*h product. z[128:192] (partitions 0..63)
  is realigned to partitions 64..127 with one SBUF->SBUF DMA per lane-pair.
  The c-gate's second M-tile (M=64) is written straight to PSUM partitions
  64..127 via tile_position, so tanh output lands aligned too.

Engine budget per step (CoreSim, steady 10240 ns): ACT 10.24us (20
activations; per-M-tile biases + the 8-bank PSUM budget make fewer/larger
activations impossible — all merge variants were measured and lost) is the
binding engine; PE 9.78us (60 fp16 matmuls, minimal for this K/M tiling);
DVE ~6.9us; GPSIMD takes the r*h products and the M2 x-half refill; SP ~6us.
Emission order puts the tanh-hi -> hi-update chain first since it frees M1
for step t+1 (the critical cycle; this broke the 10592 ns baseline period).
Step 0 skips the h-contraction (h==0); the last step drains per-lane with
y DMAs spread over SP/ACT/GPSIMD queues. fp8/DoubleRow was measured
numerically and rejected (rel err 5e-2..1.3e-1 vs the 2e-2 gate, from both
weight and rhs quantization).

The whole kernel is a single zero-idle ACT pipeline: total = first-sigmoid
start (~3.3us: parallel prologue DGE on SP/GPSIMD + a dummy activation that
hoists the 1283ns act-table load to t~0; ACT-issued DMAs before the first
activation would trigger a second table load - keep ACT clean) + 320x512ns
ACT-solid + ~4.1us drain (tail lo-updates on GPSIMD, hi chains on DVE, final
y DMAs on separate DGE queues). Total: 171158 ns CoreSim (baseline 181058).

All on-device tensors are fp16 (PSUM accumulation fp32); max rel err vs the
fp32 reference is ~1.5e-3 on HW and in CoreSim.
"""

import sys

sys.path.insert(0, "/opt/trn_rl_repo")

import numpy as np

B, T, C, H, W = 4, 16, 192, 56, 56
HH = 28          # H rows per shard
N = HH * W       # 1568 pixels per shard per step
LANE = 392       # pixels per pipeline lane (= one matmul N-chunk, one PSUM bank)
NL = N // LANE   # 4 lanes
PAIR = 2 * LANE  # 64-row ("b"-group) elementwise ops run per lane-pair
NCORES = 8

_CACHE = {}


def _build_nc(n_steps):
    from concourse import bacc
    import concourse.mybir as mybir
    import concourse.tile as tile

    F16, F32 = mybir.dt.float16, mybir.dt.float32
    AF = mybir.ActivationFunctionType

    nc = bacc.Bacc(None, target_bir_lowering=False)
    x_d = nc.dram_tensor("x", [n_steps, C, N], F16, kind="ExternalInput")
    wzr_d = nc.dram_tensor("wzr", [128, 3, 3, 128], F16, kind="ExternalInput")
    wc_d = nc.dram_tensor("wc", [128, 3, 192], F16, kind="ExternalInput")
    b_d = nc.dram_tensor("b", [128, 5], F32, kind="ExternalInput")
    y_d = nc.dram_tensor("y", [n_steps, C, N], F16, kind="ExternalOutput")

    with tile.TileContext(nc) as tc:
        with (
            tc.tile_pool(name="const", bufs=1) as constp,
            tc.tile_pool(name="state", bufs=1) as statep,
            tc.tile_pool(name="act", bufs=3) as actp,
            tc.tile_pool(name="tmp", bufs=4) as tmpp,
            tc.tile_pool(name="pszr", bufs=4, space="PSUM") as pszr,
            tc.tile_pool(name="psc", bufs=4, space="PSUM") as psc,
        ):
            Xb = [
                statep.tile([128, N], F16, name="X0a"),
                statep.tile([128, N], F16, name="X0b"),
            ]
            H0 = statep.tile([128, N], F16, name="H0")
            M1 = statep.tile([128, N], F16, name="M1")
            RH0 = statep.tile([128, N], F16, name="RH0")
            M2 = statep.tile([128, N], F16, name="M2")
            ZB = statep.tile([128, N], F16, name="ZB")
            ZRB = statep.tile([128, N], F16, name="ZRB")

            # prologue loads spread across engine DGE queues so descriptor
            # generation runs in parallel; lane-0 slices of x land first so
            # step 0's first sigmoid (which gates the zero-idle ACT pipeline)
            # starts as early as possible
            wzr = constp.tile([128, 3, 3, 128], F16)
            bb = constp.tile([128, 5], F32)
            wc = constp.tile([128, 3, 192], F16)
            # dummy activation on a tiny zeroed tile: pulls the 1283ns
            # activation-table load to t~0, off the first-sigmoid chain
            tiny = statep.tile([128, 8], F16, name="tiny")
            nc.vector.memset(tiny[:], 0.0)
            nc.scalar.activation(tiny[:, 0:1], tiny[:, 0:1], AF.Sigmoid)
            nc.sync.dma_start(wzr[:, :, 0:2, :], wzr_d[:, :, 0:2, :])
            nc.gpsimd.dma_start(M1[0:64, 0:LANE], x_d[0, 128:192, 0:LANE])
            nc.vector.memset(M1[64:128, :], 0.0)
            nc.sync.dma_start(Xb[0][:, 0:LANE], x_d[0, 0:128, 0:LANE])
            nc.gpsimd.dma_start(bb[:], b_d[:])
            nc.sync.dma_start(wzr[:, :, 2, :], wzr_d[:, :, 2, :])
            nc.gpsimd.dma_start(Xb[0][:, LANE:N], x_d[0, 0:128, LANE:N])
            nc.sync.dma_start(M1[0:64, LANE:N], x_d[0, 128:192, LANE:N])
            nc.sync.dma_start(wc[:], wc_d[:])
            nc.vector.memset(H0[:], 0.0)
            nc.gpsimd.tensor_copy(M2[0:64, :], M1[0:64, :])

            for t in range(n_steps):
                X0 = Xb[t % 2]
                if t + 1 < n_steps:
                    # X is double-buffered: prefetch can start immediately
                    nc.sync.dma_start(Xb[(t + 1) % 2][:], x_d[t + 1, 0:128, :])

                lane_z = []
                for lane in range(NL):
                    s0 = lane * LANE
                    sl = slice(s0, s0 + LANE)
                    half = lane % 2
                    p0 = (lane // 2) * PAIR
                    psl = slice(p0, p0 + PAIR)
                    zr_out = []
                    # at t=0 h==0: skip the h-contraction entirely
                    zr_srcs = ((0, H0), (2, X0), (1, M1)) if t > 0 else ((2, X0), (1, M1))
                    for m in range(3):
                        ps = pszr.tile([128, 512], F32, tag="zr")
                        # K-order defers the recurrence-written tiles:
                        # H0 (updated early by add_a), X0 (fresh DMA), M1
                        # (updated last by add_b).
                        for k, src in zr_srcs:
                            nc.tensor.matmul(
                                ps[:, 0:LANE],
                                wzr[:, k, m, :],
                                src[:, sl],
                                start=(src is zr_srcs[0][1]),
                                stop=(src is M1),
                                skip_group_check=True,
                            )
                        if m == 2:
                            nc.scalar.activation(
                                ZRB[:, sl], ps[:, 0:LANE], AF.Sigmoid,
                                bias=bb[:, m : m + 1], scale=1.0,
                            )
                        else:
                            zt = actp.tile([128, LANE], F16, tag=f"zr{m}")
                            nc.scalar.activation(
                                zt[:], ps[:, 0:LANE], AF.Sigmoid,
                                bias=bb[:, m : m + 1], scale=1.0,
                            )
                            zr_out.append(zt)
                    za, ra = zr_out
                    # rh (channels 0..127), per lane, on the (otherwise idle)
                    # GPSIMD engine to unload the DVE
                    nc.gpsimd.tensor_mul(out=RH0[:, sl], in0=ra[:], in1=H0[:, sl])
                    lane_z.append(za)
                    if half == 1:
                        # rh (channels 128..191), per lane-pair
                        nc.vector.tensor_mul(
                            out=M2[64:128, psl],
                            in0=ZRB[64:128, psl],
                            in1=M1[64:128, psl],
                        )
                        # realign z[128:192] (partitions 0..63 -> 64..127)
                        nc.sync.dma_start(ZB[64:128, psl], ZRB[0:64, psl])

                if t + 1 < n_steps:
                    # zr-phase reads of M1 are all emitted above; phase 2 only
                    # touches M1[64:128], so the x-half reload can land now.
                    nc.sync.dma_start(M1[0:64, :], x_d[t + 1, 128:192, :])

                db = None
                tail_defer = []
                for lane in range(NL):
                    s0 = lane * LANE
                    sl = slice(s0, s0 + LANE)
                    half = lane % 2
                    p0 = (lane // 2) * PAIR
                    psl = slice(p0, p0 + PAIR)
                    hf = slice(half * LANE, half * LANE + LANE)
                    if half == 0:
                        db = tmpp.tile([128, PAIR], F16, tag="db")
                    za = lane_z[lane]
                    last_pair = t + 1 == n_steps and lane >= 2
                    psa = psc.tile([128, 512], F32, tag="c")
                    psb = psc.tile([128, 512], F32, tag="c")
                    c_srcs = ((0, RH0), (2, X0), (1, M2)) if t > 0 else ((2, X0), (1, M2))
                    for k, src in c_srcs:
                        rhs = src[:, sl]
                        nc.tensor.matmul(
                            psa[:, 0:LANE], wc[:, k, 0:128], rhs,
                            start=(src is c_srcs[0][1]), stop=(src is M2),
                            skip_group_check=True,
                        )
                        nc.tensor.matmul(
                            psb[64:128, 0:LANE], wc[:, k, 128:192], rhs,
                            start=(src is c_srcs[0][1]), stop=(src is M2),
                            skip_group_check=True,
                        )
                    # tanh-hi first: it heads the chain that frees M1[64:128]
                    # for the next step's matmuls (the critical cycle)
                    nc.scalar.activation(
                        db[64:128, hf], psb[64:128, 0:LANE], AF.Tanh,
                        bias=bb[64:128, 4:5], scale=1.0,
                    )
                    if last_pair:
                        # drain-tail path: per-lane hi update + y_hi right
                        # away, on parallel engine queues
                        nc.vector.tensor_sub(
                            out=db[64:128, hf], in0=db[64:128, hf],
                            in1=M1[64:128, sl],
                        )
                        nc.vector.tensor_mul(
                            out=db[64:128, hf], in0=ZB[64:128, sl],
                            in1=db[64:128, hf],
                        )
                        nc.vector.tensor_add(
                            out=M1[64:128, sl], in0=M1[64:128, sl],
                            in1=db[64:128, hf],
                        )
                        nc.sync.dma_start(y_d[t, 128:192, sl], M1[64:128, sl])
                        ca = actp.tile([128, LANE], F16, tag="ca")
                        nc.scalar.activation(
                            ca[:], psa[:, 0:LANE], AF.Tanh,
                            bias=bb[:, 3:4], scale=1.0,
                        )
                        # both tail lo updates on GPSIMD: they start right
                        # after each lane's tanh-lo while the DVE runs only
                        # the hi chains that gate the final y_hi DMAs
                        eng = nc.gpsimd
                        da = tmpp.tile([128, LANE], F16, tag="da")
                        eng.tensor_sub(out=da[:], in0=ca[:], in1=H0[:, sl])
                        eng.tensor_mul(out=da[:], in0=za[:], in1=da[:])
                        eng.tensor_add(out=H0[:, sl], in0=H0[:, sl], in1=da[:])
                        nc.scalar.dma_start(y_d[t, 0:128, sl], H0[:, sl])
                        continue
                    ca = actp.tile([128, LANE], F16, tag="ca")
                    nc.scalar.activation(
                        ca[:], psa[:, 0:LANE], AF.Tanh, bias=bb[:, 3:4], scale=1.0
                    )
                    # hi-channel sub per lane, right after this lane's tanh-hi
                    # (db holds tanh-hi in place: db = db - h)
                    nc.vector.tensor_sub(
                        out=db[64:128, hf], in0=db[64:128, hf], in1=M1[64:128, sl]
                    )
                    if half == 1:
                        # h update (channels 128..191), per lane-pair — before
                        # the lo updates: it unblocks next step's M1 reads
                        nc.vector.tensor_mul(
                            out=db[64:128, :], in0=ZB[64:128, psl], in1=db[64:128, :]
                        )
                        nc.vector.tensor_add(
                            out=M1[64:128, psl], in0=M1[64:128, psl], in1=db[64:128, :]
                        )
                    # h update (channels 0..127), per lane: h += z*(c-h)
                    da = tmpp.tile([128, LANE], F16, tag="da")
                    nc.vector.tensor_sub(out=da[:], in0=ca[:], in1=H0[:, sl])
                    nc.vector.tensor_mul(out=da[:], in0=za[:], in1=da[:])
                    nc.vector.tensor_add(out=H0[:, sl], in0=H0[:, sl], in1=da[:])

                if t + 1 < n_steps:
                    # y output, two coalesced DMAs per step
                    nc.sync.dma_start(y_d[t, 0:128, :], H0[:, :])
                    nc.sync.dma_start(y_d[t, 128:192, :], M1[64:128, :])
                else:
                    # last step: first pair's outputs (second pair's were
                    # issued inline in the drain-tail path above)
                    nc.sync.dma_start(y_d[t, 0:128, 0:PAIR], H0[:, 0:PAIR])
                    nc.sync.dma_start(y_d[t, 128:192, 0:PAIR], M1[64:128, 0:PAIR])

                if t + 1 < n_steps:
                    # all c-phase reads of M2 are emitted; refill its x-half
                    # from M1 (same data, already on-chip) via GPSIMD
                    nc.gpsimd.tensor_copy(M2[0:64, :], M1[0:64, :])

    nc.finalize()
    return nc


def _prep_weights(w_z, w_r, w_h, b_z, b_r, b_h):
    """Host-side weight/bias packing to match the device layout."""
    wz = np.asarray(w_z, np.float32)
    wr = np.asarray(w_r, np.float32)
    wh = np.asarray(w_h, np.float32)

    def k_blocks(Wm):
        # K-tile order (H-part, mixed, X-part) matching rhs tiles (H0, M1, X0)
        return [
            Wm[:, 192:320],
            np.concatenate([Wm[:, 128:192], Wm[:, 320:384]], axis=1),
            Wm[:, 0:128],
        ]

    m_blocks = [
        wz[0:128],
        wr[0:128],
        np.concatenate([wz[128:192], wr[128:192]], axis=0),
    ]
    wzr = np.zeros((128, 3, 3, 128), np.float16)
    for m, Wm in enumerate(m_blocks):
        for k, Wk in enumerate(k_blocks(Wm)):
            wzr[:, k, m, :] = Wk.T.astype(np.float16)

    wc = np.zeros((128, 3, 192), np.float16)
    for k, Wk in enumerate(k_blocks(wh)):
        wc[:, k, :] = Wk.T.astype(np.float16)

    bpack = np.zeros((128, 5), np.float32)
    bpack[:, 0] = b_z[0:128]
    bpack[:, 1] = b_r[0:128]
    bpack[0:64, 2] = b_z[128:192]
    bpack[64:128, 2] = b_r[128:192]
    bpack[:, 3] = b_h[0:128]
    bpack[64:128, 4] = b_h[128:192]
    return wzr, wc, bpack


def _shards():
    return [(b, hh) for b in range(B) for hh in range(2)]


def kernel(**inputs):
    video = np.asarray(inputs["video"], np.float32)
    wzr, wc, bpack = _prep_weights(
        inputs["w_z"], inputs["w_r"], inputs["w_h"],
        np.asarray(inputs["b_z"], np.float32),
        np.asarray(inputs["b_r"], np.float32),
        np.asarray(inputs["b_h"], np.float32),
    )

    if "nc" not in _CACHE:
        _CACHE["nc"] = _build_nc(T)
    nc = _CACHE["nc"]

    in_maps = []
    for b, hh in _shards():
        shard = (
            video[b, :, :, hh * HH : (hh + 1) * HH, :]
            .reshape(T, C, N)
            .astype(np.float16)
        )
        in_maps.append({"x": shard, "wzr": wzr, "wc": wc, "b": bpack})

    from concourse.bass_utils import run_bass_kernel_spmd

    res = run_bass_kernel_spmd(nc, in_maps, core_ids=list(range(NCORES)))
    _CACHE["last_results"] = res

    out = np.zeros((B, T, C, H, W), np.float32)
    for ci, (b, hh) in enumerate(_shards()):
        y = res.results[ci]["y"].astype(np.float32).reshape(T, C, HH, W)
        out[b, :, :, hh * HH : (hh + 1) * HH, :] = y
    return out

